# revision 17
# baseline (speedup 1.0000x reference)
"""Trainium2 (Bass/Tile) kernel for quantized multi-head attention.

Distributed across 8 NeuronCores: tensor-parallel over heads for the
Q4_0-dequant + QKV projections + RoPE + causal attention, one small
AllToAll per batch (overlapped with later batches), then a
token-parallel output projection.

Key scheduling choices:
 - stationary-operand reuse: each x-tile LDWEIGHTS feeds the q, k and v
   matmuls; each gath-tile feeds a pair of wo panels.
 - dequant is a single broadcast-multiply per half-block (host ships
   nibbles pre-widened to int8), alternating DVE / GpSimd.
 - attention runs two chained q-chunks per head with the AV matmuls
   trailing one k-tile so the Exp latency hides under score matmuls;
   the causal diagonal mask and the softmax denominator are computed
   with tiny matmuls (mask @ I accumulate, ones^T @ ptree).
 - wo matmuls for the first panel pair interleave into batch-3's
   attention so the final AllToAll is off the critical path.
"""

import math
from dataclasses import dataclass

import numpy as np

import concourse.bass as bass
import concourse.tile as tile
from concourse import bacc, mybir, bass_isa

BF = mybir.dt.bfloat16
FP16 = mybir.dt.float16
F32 = mybir.dt.float32
I8 = mybir.dt.int8
AOP = mybir.AluOpType
AF = mybir.ActivationFunctionType


@dataclass
class Cfg:
    B: int = 4
    S: int = 1024
    D: int = 4096
    NCORES: int = 8
    SCH: int = 512   # kept for test.py compat (unused)
    QCH: int = 512   # attention q-chunk

    @property
    def T(self):
        return self.B * self.S

    @property
    def H(self):
        return self.D // 128  # total heads (head_dim 128)

    @property
    def H_LOC(self):
        return self.H // self.NCORES

    @property
    def C_SHARD(self):
        return self.H_LOC * 128  # local channels

    @property
    def SPC(self):
        return self.S // self.NCORES  # seq slice per core per batch (128)

    @property
    def TPC(self):
        return self.B * self.SPC  # tokens per core (output slice)

    @property
    def NGP(self):
        return self.D // 128  # contraction k-tiles / group-pairs per row


def build_program(cfg: Cfg):
    """Build the per-core Bass program. Returns compiled nc."""
    c = cfg
    assert c.S % c.QCH == 0 and c.QCH <= 512
    assert c.S % (128 * c.NCORES) == 0

    import concourse.tile_utils as tile_utils
    tile_utils.max_sbuf_usage = 208 * 1024

    nc = bacc.Bacc("TRN2", target_bir_lowering=False, debug=False,
                   num_devices=c.NCORES)

    OSH = c.C_SHARD      # qkv weight shard out-channels per core
    NG = 2 * c.NGP       # scale groups (of 64) per out-channel row
    ngp = c.NGP

    # ---- external I/O ----
    # x pre-tiled on host: [p, b*8+ts, g, t]
    x4_d = nc.dram_tensor("x4", [128, c.T // 128, ngp, 128], BF,
                          kind="ExternalInput")
    q8_q = nc.dram_tensor("q8q", [OSH, c.D], I8, kind="ExternalInput")
    q8_k = nc.dram_tensor("q8k", [OSH, c.D], I8, kind="ExternalInput")
    q8_v = nc.dram_tensor("q8v", [OSH, c.D], I8, kind="ExternalInput")
    q8_o = nc.dram_tensor("q8o", [c.D, c.D], I8, kind="ExternalInput")
    s4_q = nc.dram_tensor("s4q", [128, OSH // 128, NG], BF,
                          kind="ExternalInput")
    s4_k = nc.dram_tensor("s4k", [128, OSH // 128, NG], BF,
                          kind="ExternalInput")
    s4_v = nc.dram_tensor("s4v", [128, OSH // 128, NG], BF,
                          kind="ExternalInput")
    s4_o = nc.dram_tensor("s4o", [128, c.D // 128, NG], BF,
                          kind="ExternalInput")
    # rope tables, replicated over local heads; partition = s % 128
    cos4_d = nc.dram_tensor("cos4", [128, c.S // 128, c.C_SHARD], FP16,
                            kind="ExternalInput")
    sins4_d = nc.dram_tensor("sins4", [128, c.S // 128, c.C_SHARD], FP16,
                             kind="ExternalInput")
    maskl_d = nc.dram_tensor("maskl", [128, 128], BF, kind="ExternalInput")
    ident_d = nc.dram_tensor("ident", [128, 128], BF, kind="ExternalInput")
    ones_d = nc.dram_tensor("ones", [128, 1], FP16, kind="ExternalInput")
    ebias_d = nc.dram_tensor("ebias", [128, 1], F32, kind="ExternalInput")
    out_d = nc.dram_tensor("out", [c.TPC, c.D], BF, kind="ExternalOutput")

    # collective bounce buffers, one AllToAll per batch
    a2a_in = [nc.dram_tensor(f"a2a_in{b}", [c.NCORES, c.C_SHARD, c.SPC], BF)
              for b in range(c.B)]
    a2a_out = [nc.dram_tensor(f"a2a_out{b}", [c.NCORES, c.C_SHARD, c.SPC], BF)
               for b in range(c.B)]
    # dequantized+transposed first two wo panels, staged via DRAM
    wto_d = nc.dram_tensor("wto", [128, c.NGP, 1024], BF)

    inv_sqrt_d = 1.0 / math.sqrt(128.0)
    EXP_BIAS = -4.0  # exp(s/sqrt(d) - 4): keeps fp16 partial sums safe

    with tile.TileContext(nc) as tc:
        with tc.tile_pool(name="const", bufs=1) as const, \
             tc.tile_pool(name="kqv", bufs=1) as kqvp, \
             tc.tile_pool(name="attn", bufs=2) as attnp, \
             tc.tile_pool(name="scps", bufs=3, space="PSUM") as scps, \
             tc.tile_pool(name="atps", bufs=2, space="PSUM") as atps:

            # ---- constants ----
            s4q = const.tile([128, OSH // 128, NG], BF, tag="s4q")
            nc.sync.dma_start(s4q[:], s4_q[:])
            s4k = const.tile([128, OSH // 128, NG], BF, tag="s4k")
            nc.sync.dma_start(s4k[:], s4_k[:])
            s4v = const.tile([128, OSH // 128, NG], BF, tag="s4v")
            nc.sync.dma_start(s4v[:], s4_v[:])
            s4o = const.tile([128, c.D // 128, NG], BF, tag="s4o")
            nc.sync.dma_start(s4o[:], s4_o[:])
            cos4 = const.tile([128, c.S // 128, c.C_SHARD], FP16, tag="cos4")
            nc.sync.dma_start(cos4[:], cos4_d[:])
            sins4 = const.tile([128, c.S // 128, c.C_SHARD], FP16, tag="sin4")
            nc.sync.dma_start(sins4[:], sins4_d[:])
            maskl = const.tile([128, 128], BF, tag="maskl")
            nc.sync.dma_start(maskl[:], maskl_d[:])
            ident = const.tile([128, 128], BF, tag="ident")
            nc.sync.dma_start(ident[:], ident_d[:])
            ones = const.tile([128, 1], FP16, tag="ones")
            nc.sync.dma_start(ones[:], ones_d[:])
            ebias = const.tile([128, 1], F32, tag="ebias")
            nc.sync.dma_start(ebias[:], ebias_d[:])

            # ---------- dequant helper ----------
            def dequant_block(pool, q8_t, s4_t, ob, sink, eng0, eng1):
                """Dequantize one 128-oc block: q8 [128, D] i8 times
                per-group scales -> two [128, NGP/2, 128] bf16 slabs in
                natural (partition = oc) layout, handed to sink(hb, comb).
                The two halves run on eng0 / eng1."""
                q8t = pool.tile([128, c.D], I8, tag="q8", bufs=2)
                nc.sync.dma_start(q8t[:], q8_t[ob * 128:(ob + 1) * 128, :])
                q3 = q8t[:].rearrange("o (g f) -> o g f", f=64)
                half = NG // 2  # 32 groups per half-block
                for hb, eng in ((0, eng0), (1, eng1)):
                    comb = pool.tile([128, half, 64], BF, tag="comb", bufs=2)
                    eng.tensor_tensor(
                        out=comb[:],
                        in0=q3[:, hb * half:(hb + 1) * half, :],
                        in1=s4_t[:, ob, hb * half:(hb + 1) * half][:, :, None]
                        .to_broadcast([128, half, 64]),
                        op=AOP.mult)
                    sink(hb, comb)

            def dequant_to_wt(pool, wt, q8_t, s4_t, nob, wt_oc0=0,
                              eng0=None, eng1=None):
                e0 = eng0 or nc.vector
                e1 = eng1 or nc.gpsimd
                for ob in range(nob):
                    def sink(hb, comb, ob=ob):
                        g0 = hb * (ngp // 2)
                        nc.sync.dma_start(
                            out=wt[:, g0:g0 + ngp // 2,
                                   wt_oc0 + ob * 128:wt_oc0 + (ob + 1) * 128],
                            in_=comb[:].rearrange("o (gp t) f -> o gp (t f)",
                                                  t=2),
                            transpose=True)
                    dequant_block(pool, q8_t, s4_t, ob, sink, e0, e1)

            # ======= phase 1: QKV projections + attention =======
            with tc.tile_pool(name="wt", bufs=1) as wtp, \
                 tc.tile_pool(name="xt", bufs=2) as xtp, \
                 tc.tile_pool(name="dq1", bufs=2) as dq1, \
                 tc.tile_pool(name="ev", bufs=2) as evp, \
                 tc.tile_pool(name="ppsum", bufs=1, space="PSUM") as ppsum:

                wt_q = wtp.tile([128, c.NGP, OSH], BF, tag="wt_q")
                wt_k = wtp.tile([128, c.NGP, OSH], BF, tag="wt_k")
                wt_v = wtp.tile([128, c.NGP, OSH], BF, tag="wt_v")
                dequant_to_wt(dq1, wt_q, q8_q, s4q, OSH // 128)
                dequant_to_wt(dq1, wt_k, q8_k, s4k, OSH // 128)
                dequant_to_wt(dq1, wt_v, q8_v, s4v, OSH // 128)

                def load_xt(b, ts):
                    xt = xtp.tile([128, c.NGP, 128], BF, tag="xt")
                    nc.sync.dma_start(xt[:], x4_d.ap()[:, b * 8 + ts, :, :])
                    return xt

                def proj_mms(xt, mats):
                    """Emit the gp-loop with per-gp stationary reuse across
                    mats [(name, wt, psum_tile), ...].  Tail is reordered so
                    earlier mats finish a few MMs early (evacuation slack)."""
                    n = len(mats)
                    for gp in range(c.NGP - 2):
                        for _, wt_m, ps in mats:
                            nc.tensor.matmul(
                                ps[:], lhsT=xt[:, gp, :], rhs=wt_m[:, gp, :],
                                start=(gp == 0), stop=False)
                    for i, (_, wt_m, ps) in enumerate(mats):
                        for gp in (c.NGP - 2, c.NGP - 1):
                            nc.tensor.matmul(
                                ps[:], lhsT=xt[:, gp, :], rhs=wt_m[:, gp, :],
                                start=False, stop=(gp == c.NGP - 1))

                def evac_rope(b, ts, mat, ps, kt_b, qt_b, v_b):
                    st0 = ts * 128
                    if mat == "v":
                        nc.scalar.copy(out=v_b[:, ts, :], in_=ps[:])
                        return
                    # evacuate PSUM fast, rope from SBUF (fp16)
                    ev = evp.tile([128, c.C_SHARD], FP16, tag="ev" + mat,
                                  bufs=2)
                    if mat == "q":
                        nc.scalar.copy(out=ev[:], in_=ps[:])
                    else:
                        nc.vector.tensor_copy(out=ev[:], in_=ps[:])
                    roped = evp.tile([128, c.C_SHARD], FP16, tag="ro" + mat,
                                     bufs=2)
                    tmp = evp.tile([128, c.C_SHARD], FP16, tag="tm" + mat,
                                   bufs=1)
                    e3 = ev[:].rearrange("p (h d) -> p h d", d=128)
                    t3 = tmp[:].rearrange("p (h d) -> p h d", d=128)
                    s3 = sins4[:, ts, :].rearrange("p (h d) -> p h d", d=128)
                    nc.vector.tensor_tensor(
                        out=t3[:, :, 0:64], in0=e3[:, :, 64:128],
                        in1=s3[:, :, 0:64], op=AOP.mult)
                    nc.vector.tensor_tensor(
                        out=t3[:, :, 64:128], in0=e3[:, :, 0:64],
                        in1=s3[:, :, 64:128], op=AOP.mult)
                    nc.vector.tensor_tensor(
                        out=roped[:], in0=ev[:], in1=cos4[:, ts, :],
                        op=AOP.mult)
                    nc.vector.tensor_tensor(
                        out=roped[:], in0=roped[:], in1=tmp[:], op=AOP.add)
                    dst = qt_b if mat == "q" else kt_b
                    nc.sync.dma_start(
                        out=dst[:, :, st0:st0 + 128], in_=roped[:],
                        transpose=True)

                def new_psum(mat):
                    return ppsum.tile([128, OSH], F32, tag="p" + mat, bufs=1,
                                      name="ps_" + mat)

                # ---- wo staging (panels 0-1 -> DRAM) during b2/b3 slack ----
                def stage_wo_block(ob):
                    def sink(hb, comb):
                        g0 = hb * (ngp // 2)
                        wtmp = dq1.tile([128, ngp // 2, 128], BF, tag="wtmp",
                                        bufs=1)
                        nc.sync.dma_start(
                            out=wtmp[:],
                            in_=comb[:].rearrange("o (gp t) f -> o gp (t f)",
                                                  t=2),
                            transpose=True)
                        nc.sync.dma_start(
                            out=wto_d.ap()[:, g0:g0 + ngp // 2,
                                           ob * 128:(ob + 1) * 128],
                            in_=wtmp[:])
                    dequant_block(dq1, q8_o, s4o, ob, sink,
                                  nc.vector, nc.gpsimd)

                # ---------- attention ----------
                def attention_head(b, h, kt_b, qt_b, v_b):
                    """Two chained q-chunks (A: q 0-511, B: q 512-1023)
                    interleaved per k-tile; AV trails scores by one tile."""
                    atA = atps.tile([128, c.QCH], F32, tag="at")
                    atB = atps.tile([128, c.QCH], F32, tag="at")
                    ptreeA = attnp.tile([128, c.QCH], FP16, tag="ptree",
                                        bufs=2)
                    ptreeB = attnp.tile([128, c.QCH], FP16, tag="ptree",
                                        bufs=2)
                    pts = {}
                    offs = {}
                    KA, KB = 4, 8

                    def scores(chain, ki):
                        q0 = 0 if chain == "A" else 512
                        off = max(0, 128 * ki - q0)
                        offs[(chain, ki)] = off
                        diag = 128 * ki >= q0
                        sp = scps.tile([128, c.QCH], F32, tag="sc")
                        nc.tensor.matmul(
                            sp[:, off:],
                            lhsT=kt_b[:, h, ki * 128:(ki + 1) * 128],
                            rhs=qt_b[:, h, q0 + off:q0 + c.QCH],
                            start=True, stop=not diag)
                        if diag:
                            nc.tensor.matmul(
                                sp[:, off:off + 128], lhsT=maskl[:],
                                rhs=ident[:], start=False, stop=True)
                        pt = attnp.tile([128, c.QCH], FP16, tag="pt", bufs=4)
                        nc.scalar.activation(
                            out=pt[:, off:], in_=sp[:, off:], func=AF.Exp,
                            scale=inv_sqrt_d, bias=ebias[:, 0:1])
                        ptree = ptreeA if chain == "A" else ptreeB
                        if ki == 0:
                            nc.vector.tensor_copy(out=ptree[:], in_=pt[:])
                        else:
                            nc.vector.tensor_tensor(
                                out=ptree[:, off:], in0=ptree[:, off:],
                                in1=pt[:, off:], op=AOP.add)
                        pts[(chain, ki)] = pt

                    def av(chain, j, kmax):
                        off = offs[(chain, j)]
                        at = atA if chain == "A" else atB
                        nc.tensor.matmul(
                            at[:, off:],
                            lhsT=v_b[:, j, h * 128:(h + 1) * 128],
                            rhs=pts[(chain, j)][:, off:],
                            start=(j == 0), stop=(j == kmax - 1))

                    for st in range(KB + 1):
                        if st < KB:
                            scores("B", st)
                            if st < KA:
                                scores("A", st)
                        if st >= 1:
                            j = st - 1
                            av("B", j, KB)
                            if j < KA:
                                av("A", j, KA)

                    # softmax tail: z = ones^T ptree (partition sum),
                    # rz broadcast, ao = at * rz
                    for chain, at, ptree, qc in (("A", atA, ptreeA, 0),
                                                 ("B", atB, ptreeB, 1)):
                        z = scps.tile([128, c.QCH], F32, tag="sc")
                        nc.tensor.matmul(z[0:1, :], lhsT=ones[:, 0:1],
                                         rhs=ptree[:], start=True, stop=True)
                        rz = attnp.tile([1, c.QCH], F32, tag="rz", bufs=1)
                        nc.vector.reciprocal_approx_fast(rz[:], z[0:1, :])
                        rzb = attnp.tile([128, c.QCH], F32, tag="rzb",
                                         bufs=2)
                        nc.gpsimd.partition_broadcast(rzb[:], rz[:])
                        ao = attnp.tile([128, c.QCH], BF, tag="ao", bufs=2)
                        nc.vector.tensor_tensor(
                            out=ao[:], in0=at[:], in1=rzb[:], op=AOP.mult)
                        nc.sync.dma_start(
                            out=a2a_in[b][qc * 4:(qc + 1) * 4,
                                          h * 128:(h + 1) * 128, :]
                            .rearrange("r c s -> c r s"),
                            in_=ao[:])

                def do_a2a(b):
                    nc.gpsimd.collective_compute(
                        "AllToAll", AOP.bypass,
                        replica_groups=[list(range(c.NCORES))],
                        ins=[a2a_in[b].ap().opt()],
                        outs=[a2a_out[b].ap().opt()],
                    )

                # ---------- phase-1 main loop ----------
                kqv = {}
                for b in range(c.B):
                    kt_b = kqvp.tile([128, c.H_LOC, c.S], FP16, tag="kt_b")
                    qt_b = kqvp.tile([128, c.H_LOC, c.S], FP16, tag="qt_b")
                    v_b = kqvp.tile([128, c.S // 128, c.C_SHARD], FP16,
                                    tag="v_b")
                    kqv[b] = (kt_b, qt_b, v_b)

                    if b == 0:
                        # pipelined start: q-only for ts0/ts1 (waits only
                        # on wt_q), then k+v for ts0/ts1, then normal.
                        xts = [load_xt(0, 0), load_xt(0, 1)]
                        for ts in (0, 1):
                            ps = new_psum("q")
                            proj_mms(xts[ts], [("q", wt_q, ps)])
                            evac_rope(0, ts, "q", ps, kt_b, qt_b, v_b)
                        for ts in (0, 1):
                            psk, psv = new_psum("k"), new_psum("v")
                            proj_mms(xts[ts], [("k", wt_k, psk),
                                               ("v", wt_v, psv)])
                            evac_rope(0, ts, "k", psk, kt_b, qt_b, v_b)
                            evac_rope(0, ts, "v", psv, kt_b, qt_b, v_b)
                        ts_range = range(2, c.S // 128)
                    else:
                        ts_range = range(c.S // 128)

                    for ts in ts_range:
                        if b >= 2 and ts % 2 == 1:
                            stage_wo_block((b - 2) * 4 + ts // 2)
                        xt = load_xt(b, ts)
                        psq, psk, psv = (new_psum("q"), new_psum("k"),
                                         new_psum("v"))
                        proj_mms(xt, [("q", wt_q, psq), ("k", wt_k, psk),
                                      ("v", wt_v, psv)])
                        evac_rope(b, ts, "q", psq, kt_b, qt_b, v_b)
                        evac_rope(b, ts, "k", psk, kt_b, qt_b, v_b)
                        evac_rope(b, ts, "v", psv, kt_b, qt_b, v_b)

                    if b < c.B - 1:
                        for h in range(c.H_LOC):
                            attention_head(b, h, kt_b, qt_b, v_b)
                        do_a2a(b)

            # ======= phase 2: batch-3 attention + output projection =======
            with tc.tile_pool(name="gath", bufs=1) as gathp, \
                 tc.tile_pool(name="wop", bufs=1) as wopp, \
                 tc.tile_pool(name="dq2", bufs=2) as dq2, \
                 tc.tile_pool(name="osb", bufs=3) as osbp, \
                 tc.tile_pool(name="wpsum", bufs=3, space="PSUM") as wpsum:

                gath = gathp.tile([128, c.NGP, c.TPC], BF)

                def gather_b(b):
                    nc.sync.dma_start(
                        gath[:, :, b * c.SPC:(b + 1) * c.SPC],
                        a2a_out[b].ap().rearrange("s (g p) t -> p (s g) t",
                                                  p=128))

                def new_panel():
                    return wopp.tile([128, c.NGP, 512], BF, tag="wop",
                                     bufs=3, name="panel")

                # panels 0-1 from the DRAM staging area
                p0, p1 = new_panel(), new_panel()
                nc.sync.dma_start(p0[:], wto_d.ap()[:, :, 0:512])
                nc.sync.dma_start(p1[:], wto_d.ap()[:, :, 512:1024])
                for b in range(c.B - 1):
                    gather_b(b)

                def dequant_panel(panel, oc):
                    for ob in range(4):
                        def sink(hb, comb, ob=ob):
                            g0 = hb * (ngp // 2)
                            nc.sync.dma_start(
                                out=panel[:, g0:g0 + ngp // 2,
                                          ob * 128:(ob + 1) * 128],
                                in_=comb[:].rearrange(
                                    "o (gp t) f -> o gp (t f)", t=2),
                                transpose=True)
                        dequant_block(dq2, q8_o, s4o, oc * 4 + ob, sink,
                                      nc.vector, nc.gpsimd)

                def wo_store(ops, oc, tb):
                    osb = osbp.tile([128, 512], BF, tag="osb", bufs=2)
                    nc.scalar.copy(out=osb[:], in_=ops[:])
                    nc.sync.dma_start(
                        out=out_d[tb * 128:(tb + 1) * 128,
                                  oc * 512:(oc + 1) * 512],
                        in_=osb[:])

                def wo_pass(pa, pb, oca, ocb, tb):
                    """One token-block pass over a panel pair with
                    stationary reuse."""
                    opsa = wpsum.tile([128, 512], F32, tag="wo")
                    opsb = wpsum.tile([128, 512], F32, tag="wo")
                    for ct in range(c.NGP):
                        lhsT = gath[:, ct, tb * 128:(tb + 1) * 128]
                        nc.tensor.matmul(opsa[:], lhsT=lhsT, rhs=pa[:, ct, :],
                                         start=(ct == 0),
                                         stop=(ct == c.NGP - 1))
                        nc.tensor.matmul(opsb[:], lhsT=lhsT, rhs=pb[:, ct, :],
                                         start=(ct == 0),
                                         stop=(ct == c.NGP - 1))
                    wo_store(opsa, oca, tb)
                    wo_store(opsb, ocb, tb)

                def wo_single(pa, oca, tb):
                    ops = wpsum.tile([128, 512], F32, tag="wo")
                    for ct in range(c.NGP):
                        nc.tensor.matmul(
                            ops[:], lhsT=gath[:, ct, tb * 128:(tb + 1) * 128],
                            rhs=pa[:, ct, :], start=(ct == 0),
                            stop=(ct == c.NGP - 1))
                    wo_store(ops, oca, tb)

                # batch-3 attention interleaved with early wo passes on the
                # DRAM-staged panels 0-1; panel 2 dequantizes in parallel
                # (emitted after head 0 so its DVE work doesn't head-of-line
                # block the first head's softmax sums).
                kt3, qt3, v3 = kqv[c.B - 1]
                p2 = new_panel()
                for h in range(c.H_LOC):
                    attention_head(c.B - 1, h, kt3, qt3, v3)
                    if h == 0:
                        dequant_panel(p2, 2)
                    if h >= 1:
                        wo_pass(p0, p1, 0, 1, h - 1)
                do_a2a(c.B - 1)
                gather_b(c.B - 1)
                wo_single(p2, 2, 0)           # fills the collective window
                wo_pass(p0, p1, 0, 1, 3)      # needs gather-3

                # rolling pair pipeline: singles stagger each new panel's
                # dequant behind live matmul work
                p3 = new_panel()              # reuses p0's slot (now free)
                dequant_panel(p3, 3)
                p4 = new_panel()              # reuses p1's slot
                dequant_panel(p4, 4)
                wo_single(p2, 2, 1)
                wo_pass(p2, p3, 2, 3, 2)
                wo_pass(p2, p3, 2, 3, 3)
                wo_single(p3, 3, 0)
                wo_single(p3, 3, 1)
                p5 = new_panel()              # reuses p2's slot
                dequant_panel(p5, 5)
                wo_single(p4, 4, 0)
                for tb in (1, 2, 3):
                    wo_pass(p4, p5, 4, 5, tb)
                wo_single(p5, 5, 0)
                p6 = new_panel()              # reuses p3's slot
                dequant_panel(p6, 6)
                p7 = new_panel()              # reuses p4's slot
                dequant_panel(p7, 7)
                wo_single(p6, 6, 0)
                for tb in (1, 2, 3):
                    wo_pass(p6, p7, 6, 7, tb)
                wo_single(p7, 7, 0)

    nc.compile()
    return nc


# ---------------- host-side input prep ----------------

def prep_core_inputs(cfg: Cfg, x, cos_half, sin_half, mask,
                     wq_w, wq_s, wk_w, wk_s, wv_w, wv_s, wo_w, wo_s):
    """Build in_maps (list of dicts, one per core) from full inputs."""
    import ml_dtypes
    c = cfg
    bf16 = ml_dtypes.bfloat16
    HD2 = 64
    OSH = c.C_SHARD
    ngp = c.NGP

    # x pre-tiled: x4[p, b*8+ts, g, t] = x[b, ts*128+t, g*128+p]
    xr = np.asarray(x).reshape(c.B, c.S // 128, 128, ngp, 128)
    x4 = np.ascontiguousarray(xr.transpose(4, 0, 1, 3, 2)).reshape(
        128, c.T // 128, ngp, 128)

    # rope tables [128, S//128, C_SHARD] in fp16 (bf16-rounded values)
    ch = np.asarray(cos_half, np.float32)  # [S, 64]
    sh = np.asarray(sin_half, np.float32)
    cos = np.concatenate([ch, ch], axis=1).astype(bf16).astype(np.float32)
    sin = np.concatenate([sh, sh], axis=1).astype(bf16).astype(np.float32)
    sins = sin.copy()
    sins[:, :HD2] = -sin[:, :HD2]
    cos4 = np.tile(cos[:, None, :], (1, c.H_LOC, 1)).reshape(c.S, c.C_SHARD)
    sins4 = np.tile(sins[:, None, :], (1, c.H_LOC, 1)).reshape(c.S, c.C_SHARD)
    cos4 = np.ascontiguousarray(
        cos4.reshape(c.S // 128, 128, c.C_SHARD).transpose(1, 0, 2)).astype(
        np.float16)
    sins4 = np.ascontiguousarray(
        sins4.reshape(c.S // 128, 128, c.C_SHARD).transpose(1, 0, 2)).astype(
        np.float16)

    # causal-mask lhsT for the diagonal-block mask matmul
    m = np.asarray(mask, np.float32)[:128, :128]
    maskl = np.maximum(m, -1e30).astype(bf16)
    ident = np.eye(128, dtype=np.float32).astype(bf16)
    ones = np.ones((128, 1), np.float16)
    ebias = np.full((128, 1), -4.0, np.float32)

    def unpack_q8(pw, n_oc):
        """Packed Q4_0 rows -> int8 [n_oc, D] in c order."""
        w_ = np.asarray(pw).reshape(n_oc, ngp, 64)
        msb = (w_ >> 4).astype(np.int8)
        lsb = (((w_ & 15) ^ 8) - 8).astype(np.int8)
        q8 = np.stack([msb, lsb], axis=2)  # [oc, r, 2, 64]
        return np.ascontiguousarray(q8.reshape(n_oc, ngp * 128))

    def scale4(ps, n_oc):
        """Scales -> [128, n_oc//128, 2*ngp] (p, ob, g)."""
        a = np.asarray(ps).reshape(n_oc, 2 * ngp)
        return np.ascontiguousarray(
            a.reshape(n_oc // 128, 128, 2 * ngp).transpose(1, 0, 2))

    in_maps = []
    for core in range(c.NCORES):
        RPO = ngp
        r0 = core * OSH * RPO
        g0 = core * OSH * 2 * RPO
        in_maps.append({
            "x4": x4,
            "q8q": unpack_q8(np.asarray(wq_w)[r0:r0 + OSH * RPO], OSH),
            "q8k": unpack_q8(np.asarray(wk_w)[r0:r0 + OSH * RPO], OSH),
            "q8v": unpack_q8(np.asarray(wv_w)[r0:r0 + OSH * RPO], OSH),
            "q8o": unpack_q8(np.asarray(wo_w), c.D),
            "s4q": scale4(np.asarray(wq_s)[g0:g0 + OSH * 2 * RPO], OSH),
            "s4k": scale4(np.asarray(wk_s)[g0:g0 + OSH * 2 * RPO], OSH),
            "s4v": scale4(np.asarray(wv_s)[g0:g0 + OSH * 2 * RPO], OSH),
            "s4o": scale4(np.asarray(wo_s), c.D),
            "cos4": cos4,
            "sins4": sins4,
            "maskl": maskl,
            "ident": ident,
            "ones": ones,
            "ebias": ebias,
        })
    return in_maps


def unshard_output(cfg: Cfg, results):
    """results: list per core of {"out": [TPC, D]}; core r's rows are
    (b, s1) with seq slice [128r, 128(r+1)) of every batch."""
    c = cfg
    full = np.empty((c.B, c.S, c.D), dtype=np.asarray(results[0]["out"]).dtype)
    for r in range(c.NCORES):
        o = np.asarray(results[r]["out"]).reshape(c.B, c.SPC, c.D)
        full[:, r * c.SPC:(r + 1) * c.SPC, :] = o
    return full


# ======================================================================
# Self-contained kernel entry point.
# ======================================================================

_CACHE = {}


def _get_program(cfg):
    key = (cfg.B, cfg.S, cfg.D, cfg.NCORES, cfg.SCH, cfg.QCH)
    if key not in _CACHE:
        _CACHE[key] = build_program(cfg)
    return _CACHE[key]


def kernel(x, start_pos=0, cos_half=None, sin_half=None, mask=None,
           wq_w=None, wq_s=None, wk_w=None, wk_s=None,
           wv_w=None, wv_s=None, wo_w=None, wo_s=None,
           cache_k_w=None, cache_k_s=None, cache_v_w=None, cache_v_s=None,
           **_unused):
    from concourse.bass_utils import run_bass_kernel_spmd

    assert int(start_pos) == 0, "kernel specialised for start_pos == 0"
    x = np.asarray(x)
    B, S, D = x.shape
    cfg = Cfg(B=B, S=S, D=D, NCORES=8, SCH=512, QCH=512)
    # start_pos==0 with S==MAX_S, B==MAX_B: the quantized KV cache is fully
    # overwritten before use, so cache_* inputs cannot affect the output.
    in_maps = prep_core_inputs(cfg, x, cos_half, sin_half, mask,
                               wq_w, wq_s, wk_w, wk_s, wv_w, wv_s,
                               wo_w, wo_s)
    nc = _get_program(cfg)
    res = run_bass_kernel_spmd(nc, in_maps, core_ids=list(range(cfg.NCORES)))
    out = unshard_output(cfg, res.results)
    import ml_dtypes
    return out.astype(ml_dtypes.bfloat16, copy=False)
